# revision 1
# baseline (speedup 1.0000x reference)
"""ANI-style AEVComputer on 8 TRN2 NeuronCores (Bass/Tile).

Strategy
--------
Data-parallel over conformations: each of the 8 cores processes 2 of the 16
conformations (host pairs large-triple-count confs with small ones for
balance); no cross-core communication.

Angular part: instead of iterating (pair x all 64 atoms) like the previous
version (~30x wasted lanes), the host enumerates the exact (center i, j<k)
triples with both neighbors inside the angular cutoff (~1.5k per core),
ships gathered coordinates [Ri|Rj|Rk] per triple (pure indexing of the
input, no host float math on the values), and the device computes the
32 angular basis values per triple with triples laid out as
(partition, chunk) so every elementwise op covers ALL chunks in one
instruction.  Scatter to (conf-atom, species-pair) bins is one 128-wide
one-hot matmul per 128-triple chunk, PSUM-accumulated; the one-hots are
built on device from shipped integer ids via iota + is_equal.

Radial part: joint (2 conf x 64 atom) d-matrix via the 9-wide TensorE
matmul trick, 16 shifted gaussians * cutoff, one-hot species scatter by
matmul - same as before but with Sqrt instead of Ln/Exp and no diagonal
fixup (the d=0 self term contributes < 1e-6).
"""
import sys

if '/opt/trn_rl_repo' not in sys.path:
    sys.path.insert(0, '/opt/trn_rl_repo')

import numpy as np
import ml_dtypes

import concourse.bass as bass
import concourse.tile as tile
from concourse import mybir
from concourse.bass_utils import run_bass_kernel_spmd

DT = mybir.dt
AF = mybir.ActivationFunctionType
ALU = mybir.AluOpType

# ---------------- walrus compat: one sync wait per instruction ----------------


def _split_multiwaits(nc):
    n = 0
    for f in nc.m.functions:
        for bb in f.blocks:
            insts = bb.instructions
            out = []
            changed = False
            for inst in insts:
                si = inst.sync_info
                waits = list(si.on_wait) if si is not None else []
                if len(waits) > 1:
                    changed = True
                    for w in waits[:-1]:
                        n += 1
                        out.append(mybir.InstNoOp(
                            name=f"mwsplit-{n}", engine=inst.engine, ins=[], outs=[],
                            sync_info=mybir.SyncInfo(on_wait=[w], on_update=[]),
                        ))
                    inst.sync_info = mybir.SyncInfo(
                        on_wait=[waits[-1]], on_update=list(si.on_update))
                out.append(inst)
            if changed:
                insts.clear()
                insts.extend(out)
    return n


def _install_drain_patch():
    from concourse.tile import TileContext
    from concourse.vector_clock import ScopedClock

    def _patched(self, tick_clock, wait_clock):
        nc = self.nc
        drain_inst = nc.sync.drain()
        wait_clock.add_sem_waits(
            drain_inst.ins, ScopedClock({None: tick_clock.global_clock}))
        si = drain_inst.ins.sync_info
        waits = list(si.on_wait) if si else []
        if len(waits) > 1:
            drain_inst.ins.sync_info = mybir.SyncInfo(
                on_wait=waits[:1], on_update=[])
            engs = [nc.vector, nc.scalar, nc.gpsimd, nc.tensor, nc.sync]
            for idx, wt in enumerate(waits[1:]):
                e = engs[idx % len(engs)]
                nop = e.nop(nofuse=True)
                nop.ins.sync_info = mybir.SyncInfo(on_wait=[wt], on_update=[])
        nc.all_engine_barrier()
        assert self.sems is not None
        popped = nc._tile_sem_poison_stack.pop()
        assert popped is self._sem_poison
        nc.clear_and_free_semaphores(list(self.sems.allocated().values()))
        nc.all_engine_barrier()

    TileContext._drain_and_barrier = _patched


_install_drain_patch()

# ---------------- problem constants ----------------
RCR, RCA = 5.2, 3.5
SHF_R = (0.9 + 0.26875 * np.arange(16)).astype(np.float64)
SHF_A = np.array([0.9, 1.55, 2.2, 2.85], np.float64)
SHF_Z = (np.pi / 16 + (np.pi / 8) * np.arange(8)).astype(np.float64)
NSP = 4
C, A = 16, 64
A2 = 2 * A
NCORES, CPC = 8, 2

_tbl = np.zeros((NSP, NSP), np.int64)
_k = 0
for _a in range(NSP):
    for _b in range(_a, NSP):
        _tbl[_a, _b] = _tbl[_b, _a] = _k
        _k += 1
NPAIR_T = _k                  # 10

# lane-constant table columns
_LN_SHFA2 = 0                 # 4: 2*SHF_A
_LN_CZ = 4                    # 8: 0.475*cos(SHF_Z)
_LN_SZ = 12                   # 8: 0.5*sin(SHF_Z)
_LN_SHFR = 20                 # 16: SHF_R
_LN_PI2 = 36                  # 1: pi/2
_LN_LN2 = 37                  # 1: ln(2)
_LN_N = 38

_NC_CACHE = {}


def _build(NCH):
    """Per-core Bass graph; NCH = number of 128-triple chunks."""
    nc = bass.Bass("TRN2", target_bir_lowering=False, debug=False)

    coords = nc.declare_dram_parameter("coords", [CPC, A, 3], DT.float32, isOutput=False)
    rjk = nc.declare_dram_parameter("rjk", [A2, NCH, 12], DT.float32, isOutput=False)
    meta = nc.declare_dram_parameter("meta", [A2, 2, NCH], DT.float32, isOutput=False)
    lanes = nc.declare_dram_parameter("lanes", [A2, _LN_N], DT.float32, isOutput=False)
    ohs = nc.declare_dram_parameter("ohs", [A2, 2 * NSP], DT.bfloat16, isOutput=False)
    out = nc.declare_dram_parameter("out", [CPC, A, 64], DT.float32, isOutput=True)
    outa = nc.declare_dram_parameter("outa", [CPC, A, 320], DT.bfloat16, isOutput=True)

    with tile.TileContext(nc) as tc:
        with tc.tile_pool(name="sb", bufs=1) as sb, \
             tc.tile_pool(name="ps", bufs=1, space="PSUM") as ps:

            # ---- latency-critical small DMAs first (keep their queue clear) --
            cts = []
            for cc in range(CPC):
                ct = sb.tile([3, A], DT.float32, tag="ct", name=f"ct{cc}")
                nc.sync.dma_start(ct[:], coords[cc].rearrange("a k -> k a"))
                cts.append(ct)
            # big input tables on separate engines/queues
            iotaF = sb.tile([A2, 128], DT.float32)
            nc.gpsimd.iota(iotaF[:], [[1, 128]], channel_multiplier=0,
                           allow_small_or_imprecise_dtypes=True)
            rjk_sb = sb.tile([A2, NCH, 12], DT.float32)
            h1 = (NCH + 1) // 2
            nc.scalar.dma_start(rjk_sb[:, 0:h1, :], rjk[:, 0:h1, :])
            meta_sb = sb.tile([A2, 2, NCH], DT.float32)
            nc.gpsimd.dma_start(meta_sb[:], meta[:])
            nc.gpsimd.dma_start(rjk_sb[:, h1:NCH, :], rjk[:, h1:NCH, :])
            lanes_sb = sb.tile([A2, _LN_N], DT.float32)
            nc.sync.dma_start(lanes_sb[:], lanes[:])
            ohs_sb = sb.tile([A2, 2 * NSP], DT.bfloat16)
            nc.gpsimd.dma_start(ohs_sb[:], ohs[:])
            # radial shift constants, materialized (fast contiguous operands)
            shfrq = sb.tile([A2, 16, A], DT.float32)
            for rr in range(16):
                nc.gpsimd.memset(shfrq[:, rr, :], 4.0 * SHF_R[rr])

            def lane(c0, n, w):
                return lanes_sb[:, c0:c0 + n].rearrange(
                    "p (c k) -> p c k", c=1).broadcast_to([A2, w, n])

            # ---------- radial phase (joint 2 confs) ----------
            NSQ = A + 3 * NCH
            sqin = sb.tile([A2, NSQ], DT.float32)
            ones3 = sb.tile([3, A], DT.float32)
            nc.gpsimd.memset(ones3[:], 1.0)
            for cc in range(CPC):
                ct = cts[cc]
                sq3 = sb.tile([3, A], DT.float32, tag=f"sq3{cc}", name=f"sq3{cc}")
                m2ct = sb.tile([3, A], DT.float32, tag=f"m2ct{cc}", name=f"m2ct{cc}")
                nc.scalar.square(sq3[:], ct[:])
                nc.vector.tensor_scalar(m2ct[:], ct[:], -2.0, None, ALU.mult)
                dsqp = ps.tile([A, A], DT.float32, tag=f"dsq{cc}", name=f"dsq{cc}")
                nc.tensor.matmul(dsqp[:], sq3[:], ones3[:], start=True, stop=False)
                nc.tensor.matmul(dsqp[:], ones3[:], sq3[:], start=False, stop=False)
                nc.tensor.matmul(dsqp[:], ct[:], m2ct[:], start=False, stop=True)
                nc.vector.tensor_scalar(sqin[A * cc:A * (cc + 1), 0:A], dsqp[:],
                                        0.0, None, ALU.max)
            # ---------- triple stream: geometry ----------
            v12 = sb.tile([A2, NCH, 2, 3], DT.float32)
            nc.vector.tensor_tensor(
                v12[:].rearrange("p c u x -> p c (u x)"),
                rjk_sb[:, :, 0:6], rjk_sb[:, :, 6:12], ALU.subtract)
            ohci = sb.tile([A2, NCH, 128], DT.bfloat16)
            nc.vector.tensor_tensor(
                ohci[:],
                meta_sb[:, 0, :].rearrange("p (c m) -> p c m", m=1
                                           ).broadcast_to([A2, NCH, 128]),
                iotaF[:].rearrange("p (c m) -> p c m", c=1
                                   ).broadcast_to([A2, NCH, 128]),
                ALU.is_equal)
            ohsp = sb.tile([A2, NCH, NPAIR_T], DT.bfloat16)
            nc.vector.tensor_tensor(
                ohsp[:],
                meta_sb[:, 1, :].rearrange("p (c m) -> p c m", m=1
                                           ).broadcast_to([A2, NCH, NPAIR_T]),
                iotaF[:, 0:NPAIR_T].rearrange("p (c m) -> p c m", c=1
                                              ).broadcast_to([A2, NCH, NPAIR_T]),
                ALU.is_equal)
            sq6 = sb.tile([A2, NCH, 2, 3], DT.float32)
            nc.vector.tensor_tensor(sq6[:], v12[:], v12[:], ALU.mult)
            dm3 = sb.tile([A2, NCH, 3], DT.float32)
            nc.gpsimd.tensor_tensor(dm3[:], v12[:, :, 0, :], v12[:, :, 1, :],
                                    ALU.mult)
            dq2 = sqin[:, A:A + 2 * NCH].rearrange("p (c u) -> p c u", u=2)
            nc.vector.tensor_reduce(dq2, sq6[:], mybir.AxisListType.X, ALU.add)
            dot = sb.tile([A2, NCH, 1], DT.float32)
            nc.vector.tensor_reduce(dot[:], dm3[:], mybir.AxisListType.X, ALU.add)
            # sin^2-term argument before any sqrt: usq = dot^2/(dq1*dq2)
            dqp = sb.tile([A2, NCH, 1], DT.float32)
            nc.gpsimd.tensor_tensor(dqp[:], dq2[:, :, 0:1], dq2[:, :, 1:2], ALU.mult)
            dot2 = sb.tile([A2, NCH, 1], DT.float32)
            nc.gpsimd.tensor_tensor(dot2[:], dot[:], dot[:], ALU.mult)
            rcpq = sb.tile([A2, NCH, 1], DT.float32)
            nc.vector.reciprocal(rcpq[:], dqp[:])
            usq = sb.tile([A2, NCH, 1], DT.float32)
            nc.vector.tensor_tensor(usq[:], dot2[:], rcpq[:], ALU.mult)
            nc.vector.tensor_scalar(
                sqin[:, A + 2 * NCH:NSQ].rearrange("p (c u) -> p c u", u=1),
                usq[:], -0.9025, 1.0, ALU.mult, ALU.add)

            # ONE Sqrt for d-matrix, pair distances and sin-term
            dall = sb.tile([A2, NSQ], DT.float32)
            nc.scalar.activation(dall[:], sqin[:], AF.Sqrt)
            d_t = dall[:, 0:A]
            d2l = dall[:, A:A + 2 * NCH].rearrange("p (c u) -> p c u", u=2)
            ss = dall[:, A + 2 * NCH:NSQ].rearrange("p (c u) -> p c u", u=1)

            # cutoff masks + ONE Sin for both radii (scale folded into args)
            mskR = sb.tile([A2, A], DT.float32)
            nc.vector.tensor_scalar(mskR[:], d_t, RCR, None, ALU.is_le)
            msk2 = sb.tile([A2, NCH, 2], DT.float32)
            nc.vector.tensor_scalar(msk2[:], d2l, RCA, None, ALU.is_le)
            NSN = A + 2 * NCH
            sinin = sb.tile([A2, NSN], DT.float32)
            nc.vector.tensor_scalar(sinin[:, 0:A], d_t, RCR * 1.01, np.pi / RCR,
                                    ALU.min, ALU.mult)
            nc.vector.tensor_scalar(
                sinin[:, A:NSN].rearrange("p (c u) -> p c u", u=2),
                d2l, RCA * 1.01, np.pi / RCA, ALU.min, ALU.mult)
            snall = sb.tile([A2, NSN], DT.float32)
            nc.scalar.activation(snall[:], sinin[:], AF.Sin,
                                 bias=lanes_sb[:, _LN_PI2:_LN_PI2 + 1],
                                 scale=-1.0)
            fchR = sb.tile([A2, A], DT.float32)
            nc.vector.tensor_scalar(fchR[:], snall[:, 0:A], 0.125, 0.125,
                                    ALU.mult, ALU.add)
            fcR = sb.tile([A2, A], DT.float32)
            nc.gpsimd.tensor_tensor(fcR[:], fchR[:], mskR[:], ALU.mult)
            fch2 = sb.tile([A2, NCH, 2], DT.float32)
            nc.vector.tensor_scalar(
                fch2[:], snall[:, A:NSN].rearrange("p (c u) -> p c u", u=2),
                0.5, 0.5, ALU.mult, ALU.add)
            g2 = sb.tile([A2, NCH, 2], DT.float32)
            nc.gpsimd.tensor_tensor(g2[:], fch2[:], msk2[:], ALU.mult)
            g = sb.tile([A2, NCH, 1], DT.float32)
            nc.vector.tensor_tensor(g[:], g2[:, :, 0:1], g2[:, :, 1:2], ALU.mult)

            prod = sb.tile([A2, NCH, 1], DT.float32)
            nc.gpsimd.tensor_tensor(prod[:], d2l[:, :, 0:1], d2l[:, :, 1:2], ALU.mult)
            tsum = sb.tile([A2, NCH, 1], DT.float32)
            nc.gpsimd.tensor_tensor(tsum[:], d2l[:, :, 0:1], d2l[:, :, 1:2], ALU.add)
            rcp = sb.tile([A2, NCH, 1], DT.float32)
            nc.vector.reciprocal(rcp[:], prod[:])
            u = sb.tile([A2, NCH, 1], DT.float32)
            nc.vector.tensor_tensor(u[:], dot[:], rcp[:], ALU.mult)

            # f2 path (small Exp; radial gaussians get their own later)
            ts2 = sb.tile([A2, NCH, 1], DT.float32)
            nc.vector.tensor_scalar(ts2[:], tsum[:], np.sqrt(2.0), None, ALU.mult)
            am = sb.tile([A2, NCH, 4], DT.float32)
            nc.gpsimd.tensor_tensor(
                am[:], ts2[:].broadcast_to([A2, NCH, 4]),
                lane(_LN_SHFA2, 4, NCH), ALU.subtract)
            amsq = sb.tile([A2, NCH, 4], DT.float32)
            nc.vector.tensor_tensor(amsq[:], am[:], am[:], ALU.mult)
            hc = sb.tile([A2, NCH, 8], DT.float32)
            nc.vector.tensor_tensor(hc[:], u[:].broadcast_to([A2, NCH, 8]),
                                    lane(_LN_CZ, 8, NCH), ALU.mult)
            hs = sb.tile([A2, NCH, 8], DT.float32)
            nc.gpsimd.tensor_tensor(hs[:], ss.broadcast_to([A2, NCH, 8]),
                                    lane(_LN_SZ, 8, NCH), ALU.mult)
            nc.vector.tensor_tensor(hc[:], hc[:], hs[:], ALU.add)
            nc.vector.tensor_scalar(hc[:], hc[:], 0.5, 0.0, ALU.add, ALU.max)

            f2 = sb.tile([A2, NCH, 4], DT.float32)
            nc.scalar.activation(f2[:], amsq[:], AF.Exp, scale=-1.0)
            lnh = sb.tile([A2, NCH, 8], DT.float32)
            nc.scalar.activation(lnh[:], hc[:], AF.Ln)
            f1 = sb.tile([A2, NCH, 8], DT.bfloat16)
            nc.scalar.activation(f1[:], lnh[:], AF.Exp, scale=32.0,
                                 bias=lanes_sb[:, _LN_LN2:_LN_LN2 + 1])

            f2g = sb.tile([A2, NCH, 4], DT.bfloat16)
            nc.vector.tensor_tensor(f2g[:], f2[:], g[:].broadcast_to([A2, NCH, 4]),
                                    ALU.mult)
            at = sb.tile([A2, NCH, 32], DT.bfloat16)
            nc.vector.tensor_tensor(
                at[:].rearrange("p c (a z) -> p c a z", a=4),
                f1[:].rearrange("p c (a z) -> p c a z", a=1
                                ).broadcast_to([A2, NCH, 4, 8]),
                f2g[:].rearrange("p c (a z) -> p c a z", z=1
                                 ).broadcast_to([A2, NCH, 4, 8]),
                ALU.mult)

            # species-pair expansion + scatter, split in halves so the
            # matmuls overlap the second half's build and the radial tail
            at320 = sb.tile([A2, NCH, NPAIR_T, 32], DT.bfloat16)
            angp = ps.tile([A2, NPAIR_T * 32], DT.float32, tag="angp")
            hh = (NCH + 1) // 2

            def build320(c0, c1):
                nc.vector.tensor_tensor(
                    at320[:, c0:c1],
                    at[:, c0:c1].rearrange("p c (s w) -> p c s w", s=1
                                           ).broadcast_to([A2, c1 - c0, NPAIR_T, 32]),
                    ohsp[:, c0:c1].rearrange("p c (s w) -> p c s w", w=1
                                             ).broadcast_to([A2, c1 - c0, NPAIR_T, 32]),
                    ALU.mult)

            build320(0, hh)
            for ch in range(hh):
                nc.tensor.matmul(angp[:], ohci[:, ch, :], at320[:, ch],
                                 start=(ch == 0), stop=False)

            # radial tail on otherwise-idle slots while the matmuls run
            d4 = sb.tile([A2, A], DT.float32)
            nc.vector.tensor_scalar(d4[:], d_t, 4.0, None, ALU.mult)
            rsub = sb.tile([A2, 16, A], DT.float32)
            nc.vector.tensor_tensor(
                rsub[:],
                d4[:].rearrange("p (r i) -> p r i", r=1).broadcast_to([A2, 16, A]),
                shfrq[:], ALU.subtract)
            rsq = sb.tile([A2, 16, A], DT.float32)
            nc.vector.tensor_tensor(rsq[:], rsub[:], rsub[:], ALU.mult)
            rte = sb.tile([A2, 16, A], DT.float32)
            nc.scalar.activation(rte[:], rsq[:], AF.Exp, scale=-1.0)

            build320(hh, NCH)
            for ch in range(hh, NCH):
                nc.tensor.matmul(angp[:], ohci[:, ch, :], at320[:, ch],
                                 start=False, stop=(ch == NCH - 1))

            rtf = sb.tile([A2, 16, A], DT.bfloat16)
            nc.gpsimd.tensor_tensor(
                rtf[:], rte[:],
                fcR[:].rearrange("p (r i) -> p r i", r=1).broadcast_to([A2, 16, A]),
                ALU.mult)
            radsb = sb.tile([2 * NSP, A, 16], DT.float32)
            for half in range(2):
                radp = ps.tile([2 * NSP, 8, A], DT.float32, tag="radp",
                               name=f"radp{half}")
                nc.tensor.matmul(radp[:], ohs_sb[:], rtf[:, 8 * half:8 * (half + 1), :],
                                 start=True, stop=True)
                dst = radsb[:, :, 8 * half:8 * (half + 1)]
                src = radp[:].rearrange("s z i -> s i z")
                if half == 0:
                    nc.scalar.copy(dst, src)
                else:
                    nc.vector.tensor_copy(dst, src)
            for cc in range(CPC):
                nc.sync.dma_start(
                    out[cc].rearrange("i (s r) -> s i r", s=NSP),
                    radsb[NSP * cc:NSP * (cc + 1)])

            angsb = sb.tile([A2, NPAIR_T * 32], DT.bfloat16)
            nc.scalar.copy(angsb[:, 0:160], angp[:, 0:160])
            nc.vector.tensor_copy(angsb[:, 160:320], angp[:, 160:320])
            engs = [nc.sync, nc.gpsimd, nc.scalar, nc.sync]
            k = 0
            for cc in range(CPC):
                for rh in range(2):
                    r0 = 32 * rh
                    engs[k % 4].dma_start(
                        outa[cc, r0:r0 + 32, :],
                        angsb[A * cc + r0:A * cc + r0 + 32, :])
                    k += 1

    _split_multiwaits(nc)
    return nc


# ---------------- host side ----------------

def _prep(species, coordinates):
    sp = np.clip(np.asarray(species).astype(np.int64), 0, NSP - 1)
    co = np.ascontiguousarray(np.asarray(coordinates), dtype=np.float32)
    d2 = ((co[:, :, None, :].astype(np.float64) - co[:, None, :, :]) ** 2).sum(-1)
    D = np.sqrt(d2)
    for c in range(C):
        np.fill_diagonal(D[c], 1e9)
    near = D < (RCA + 0.02)

    # enumerate (center, j<k) triples per conformation
    tri = []
    for c in range(C):
        Is, Js, Ks = [], [], []
        for i in range(A):
            nz = np.nonzero(near[c, i])[0]
            m = nz.size
            if m >= 2:
                jj, kk = np.triu_indices(m, k=1)
                Is.append(np.full(jj.size, i, np.int64))
                Js.append(nz[jj])
                Ks.append(nz[kk])
        if Is:
            tri.append((np.concatenate(Is), np.concatenate(Js), np.concatenate(Ks)))
        else:
            tri.append((np.zeros(0, np.int64),) * 3)

    counts = np.array([t[0].size for t in tri])
    order = np.argsort(-counts)
    confs = [(int(order[k]), int(order[15 - k])) for k in range(NCORES)]

    NCH = max(1, int(np.ceil(max(counts[ca] + counts[cb] for ca, cb in confs) / 128)))
    KT = NCH * 128

    lane_row = np.zeros(_LN_N, np.float64)
    lane_row[_LN_SHFA2:_LN_SHFA2 + 4] = 2.0 * np.sqrt(2.0) * SHF_A
    lane_row[_LN_CZ:_LN_CZ + 8] = 0.475 * np.cos(SHF_Z)
    lane_row[_LN_SZ:_LN_SZ + 8] = 0.5 * np.sin(SHF_Z)
    lane_row[_LN_SHFR:_LN_SHFR + 16] = SHF_R
    lane_row[_LN_PI2] = np.pi / 2
    lane_row[_LN_LN2] = np.log(2.0)
    lanes_t = np.tile(lane_row.astype(np.float32), (A2, 1))

    pad_rjk = np.array([0, 0, 0, 0, 0, 0, 60, 0, 0, 0, 60, 0], np.float32)

    per_core = []
    for k in range(NCORES):
        ca, cb = confs[k]
        rjk_l, ci_l, spid_l = [], [], []
        for cc, c in enumerate((ca, cb)):
            I, J, K = tri[c]
            if I.size:
                rjk_l.append(np.concatenate(
                    [co[c, I], co[c, I], co[c, J], co[c, K]], axis=1))
                ci_l.append(A * cc + I)
                spid_l.append(_tbl[sp[c, J], sp[c, K]])
        T = sum(x.size for x in ci_l)
        rjk_f = np.full((KT, 12), 0, np.float32)
        rjk_f[:] = pad_rjk
        ci_f = np.zeros(KT, np.float32)
        spid_f = np.zeros(KT, np.float32)
        if T:
            rjk_f[:T] = np.concatenate(rjk_l, axis=0)
            ci_f[:T] = np.concatenate(ci_l).astype(np.float32)
            spid_f[:T] = np.concatenate(spid_l).astype(np.float32)
        # triple t = ch*128 + p  ->  tile [p, ch]
        rjk_t = rjk_f.reshape(NCH, 128, 12).transpose(1, 0, 2)
        meta_t = np.stack([ci_f.reshape(NCH, 128).T,
                           spid_f.reshape(NCH, 128).T], axis=1)  # (128, 2, NCH)

        ohsv = np.zeros((A2, 2 * NSP), np.float32)
        for cc, c in enumerate((ca, cb)):
            ohsv[A * cc:A * (cc + 1), NSP * cc:NSP * (cc + 1)] = (
                sp[c][:, None] == np.arange(NSP))
        per_core.append({
            "coords": np.ascontiguousarray(np.stack([co[ca], co[cb]])),
            "rjk": np.ascontiguousarray(rjk_t),
            "meta": np.ascontiguousarray(meta_t),
            "lanes": lanes_t,
            "ohs": ohsv.astype(ml_dtypes.bfloat16),
        })
    return NCH, per_core, confs


def _run(species, coordinates, trace=False):
    NCH, in_maps, confs = _prep(species, coordinates)
    if NCH not in _NC_CACHE:
        _NC_CACHE[NCH] = _build(NCH)
    nc = _NC_CACHE[NCH]
    res = run_bass_kernel_spmd(nc, in_maps, core_ids=list(range(NCORES)), trace=trace)
    full = np.empty((C, A, 384), np.float32)
    for k in range(NCORES):
        orad = res.results[k]["out"]
        oang = np.asarray(res.results[k]["outa"]).astype(np.float32)
        for cc in range(CPC):
            full[confs[k][cc], :, 0:64] = orad[cc]
            full[confs[k][cc], :, 64:] = oang[cc]
    return full, res


def kernel(species, coordinates):
    out, _ = _run(species, coordinates, trace=False)
    return out

